# revision 22
# baseline (speedup 1.0000x reference)
"""Multi-head attention (B=2, N=2048, C=1024, H=16, D=64) on 8 trn2 cores.

Sharding: core c -> (batch b = c//4, head-group g = c%4 covering 4 heads).
Tensor-parallel over heads: Wq/Wk/Wv split column-wise, Wo row-wise; the
4 partial outputs per batch are summed on the host (+ bias).

Device layout trick: the host feeds activations TRANSPOSED ([C, seq]) so
every matmul on chip has its contraction dim on partitions with no
on-chip transposes:
  QT/KT panels [f, seq]  (projection outputs, transposed orientation)
  VP panel     [seq, f]  (natural orientation, +ones column per head)
  S^T  = Kh @ QhT        [sk, sq]  (d=64 contraction, 2-head row-packed)
  P^T  = exp(S^T * scale)          (ScalarE, reads PSUM directly)
  O'^T = [Vh|1]T-style   [65, sq]  (row 64 = softmax denominator)
  Y^T  = Wo^T @ (O^T/den)[o, seq]
Inputs stream in as fp16 (half the DMA of fp32, 8x the mantissa of
bf16); all matmuls run fp16 with fp32 PSUM accumulation.
"""

import os
import sys

import numpy as np

sys.path.insert(0, "/opt/trn_rl_repo")

import concourse.bacc as bacc
import concourse.bass as bass
import concourse.tile as tile
from concourse import mybir
from concourse.bass_utils import run_bass_kernel_spmd

F32 = mybir.dt.float32
F32R = mybir.dt.float32r
F16 = mybir.dt.float16

B = 2
SEQ = 2048
C = 1024
NH = 4          # heads per core
D = 64
FH = NH * D     # 256: feature slice per core
SCALE = D ** -0.5

N_CORES = 8
CCN = C // 128      # 8 contraction chunks
SQN = SEQ // 512    # 4 query chunks
SKN = SEQ // 128    # 16 key chunks

LAST_RESULTS = None  # stash for test harness introspection


def build_kernel(tc, qT, kT, vT, wq, wk, wv, wo, yT):
    nc = tc.nc

    with (
        tc.tile_pool(name="weights", bufs=1) as wpool,
        tc.tile_pool(name="panels", bufs=1) as panels,
        tc.tile_pool(name="xin", bufs=11) as xpool,
        tc.tile_pool(name="ptile", bufs=6) as ppool,
        tc.tile_pool(name="otile", bufs=3) as opool,
        tc.tile_pool(name="ytile", bufs=4) as ypool,
        tc.tile_pool(name="small", bufs=4) as small,
    ):
        # ---- resident weights ----
        wq_sb = wpool.tile([128, CCN, FH], F16, name="wq_sb", tag="wq")
        wk_sb = wpool.tile([128, CCN, FH], F16, name="wk_sb", tag="wk")
        wv_sb = wpool.tile([128, CCN, FH], F16, name="wv_sb", tag="wv")
        wo_sb = wpool.tile([128, 2, C], F16, name="wo_sb", tag="wo")
        nc.sync.dma_start(out=wv_sb, in_=wv[:, :].rearrange("(n p) m -> p n m", p=128))
        nc.sync.dma_start(out=wk_sb, in_=wk[:, :].rearrange("(n p) m -> p n m", p=128))

        # ---- persistent activation panels ----
        qt_sb = panels.tile([128, 2, SEQ], F16, name="qt_sb", tag="qt")   # [p, fc, sq] = QT
        kt_sb = panels.tile([128, 2, SEQ], F16, name="kt_sb", tag="kt")   # [p, fc, sk] = KT
        vp_sb = panels.tile([128, SKN, NH, D + 1], F16, name="vp_sb", tag="vp")  # V' natural
        nc.vector.memset(vp_sb[:, :, :, D:D + 1], 1.0)

        # ---- projections (PSUM: 8 banks of [128,512] accumulators) ----
        with tc.tile_pool(name="ps_proj", bufs=8, space="PSUM") as ps_proj:
            # V first: its 8 vT chunks stream in while its matmuls (and then
            # K's) fill the PE. All 8 stay resident so each skc accumulates
            # in its own PSUM bank (start=True zeroes a whole bank).
            vx = []
            for cc in range(CCN):
                xin = xpool.tile([128, SEQ], F16, name="xin", tag="xin")
                nc.sync.dma_start(out=xin, in_=vT[cc * 128:(cc + 1) * 128, :])
                vx.append(xin)
            for skc in range(SKN):
                vacc = ps_proj.tile([128, 256], F32, name="vacc", tag="pacc")
                for cc in range(CCN):
                    nc.tensor.matmul(
                        out=vacc,
                        lhsT=vx[cc][:, skc * 128:(skc + 1) * 128],
                        rhs=wv_sb[:, cc, :],
                        start=(cc == 0),
                        stop=(cc == CCN - 1),
                    )
                v_dst = vp_sb[:, skc, :, 0:D]
                v_src = vacc.rearrange("p (h d) -> p h d", h=NH)
                if skc % 2 == 0:
                    nc.vector.tensor_copy(out=v_dst, in_=v_src)
                else:
                    nc.scalar.copy(out=v_dst, in_=v_src)

            # K then Q: out panel [f, seq]; stationary = W chunk, moving = xT.
            first = True
            for name, src, w_sb, dst in (
                ("k", kT, wk_sb, kt_sb),
                ("q", qT, wq_sb, qt_sb),
            ):
                acc = {}
                for cc in range(CCN):
                    xin = xpool.tile([128, SEQ], F16, name="xin", tag="xin")
                    nc.sync.dma_start(out=xin, in_=src[cc * 128:(cc + 1) * 128, :])
                    for fc in range(2):
                        for sqc in range(SQN):
                            if cc == 0:
                                acc[(fc, sqc)] = ps_proj.tile(
                                    [128, 512], F32, name="pacc", tag="pacc"
                                )
                            nc.tensor.matmul(
                                out=acc[(fc, sqc)],
                                lhsT=w_sb[:, cc, fc * 128:(fc + 1) * 128],
                                rhs=xin[:, sqc * 512:(sqc + 1) * 512],
                                start=(cc == 0),
                                stop=(cc == CCN - 1),
                            )
                for i, (fc, sqc) in enumerate(
                    (fc, sqc) for fc in range(2) for sqc in range(SQN)
                ):
                    dst_ap = dst[:, fc, sqc * 512:(sqc + 1) * 512]
                    if i % 2 == 0:
                        nc.vector.tensor_copy(out=dst_ap, in_=acc[(fc, sqc)])
                    else:
                        nc.scalar.copy(out=dst_ap, in_=acc[(fc, sqc)])
                if first:
                    # queue Q/Wo weights behind the V+K input streams
                    nc.sync.dma_start(
                        out=wq_sb,
                        in_=wq[:, :].rearrange("(n p) m -> p n m", p=128),
                    )
                    nc.sync.dma_start(
                        out=wo_sb,
                        in_=wo[:, :].rearrange("(n p) m -> p n m", p=128),
                    )
                    first = False
                else:
                    # keep the PE busy while the last drains release the
                    # projection PSUM pool (a >3.4us idle would re-throttle
                    # the clock via HAM for the start of attention)
                    warm = ps_proj.tile([128, 512], F32, name="warm",
                                        tag="pacc")
                    for i in range(12):
                        nc.tensor.matmul(
                            out=warm,
                            lhsT=w_sb[:, 0, 0:128],
                            rhs=xin[:, 0:512],
                            start=True,
                            stop=True,
                        )

        # ---- attention + output projection ----
        # Per (sqc, hp) the skc loop is software-pipelined so the PE never
        # sits behind the ACT exp in its in-order queue:
        #   iter k emits  S(k) -> exp(k) -> PV(k-1)
        # and the previous query-chunk's output projection is drip-fed into
        # the same stream (one oc unit per iter) to keep the PE dense while
        # the ACT-bound exp stream is the critical path.
        with (
            tc.tile_pool(name="ps_s", bufs=2, space="PSUM") as ps_s,
            tc.tile_pool(name="ps_o", bufs=2, space="PSUM") as ps_o,
            tc.tile_pool(name="ps_y", bufs=2, space="PSUM") as ps_y,
        ):
            def yproj_steps(ot_tile, sq_slice, oc, tail=False):
                state = {}
                # in the tail the attention accumulators are retired, so
                # alternate Y groups into the ps_o pool for 4-deep rotation
                pool, tag = ((ps_o, "oacc") if tail and oc % 2 == 1
                             else (ps_y, "yacc"))

                def mm0():
                    state["y_ps"] = pool.tile([128, 512], F32, name="yacc",
                                              tag=tag)
                    nc.tensor.matmul(
                        out=state["y_ps"],
                        lhsT=wo_sb[:, 0, oc * 128:(oc + 1) * 128],
                        rhs=ot_tile[:, 0, :],
                        start=True,
                        stop=False,
                    )

                def mm1():
                    nc.tensor.matmul(
                        out=state["y_ps"],
                        lhsT=wo_sb[:, 1, oc * 128:(oc + 1) * 128],
                        rhs=ot_tile[:, 1, :],
                        start=False,
                        stop=True,
                    )
                    y_sb = ypool.tile([128, 512], F32, name="y", tag="y")
                    if tail and oc % 2 == 1:
                        nc.scalar.copy(out=y_sb, in_=y_ps_ref())
                    else:
                        nc.vector.tensor_copy(out=y_sb, in_=y_ps_ref())
                    nc.sync.dma_start(
                        out=yT[oc * 128:(oc + 1) * 128, sq_slice], in_=y_sb
                    )

                def y_ps_ref():
                    return state["y_ps"]

                return [mm0, mm1]

            pending = []  # deferred Yproj units from the previous sqc
            for sqc in range(SQN):
                sq = slice(sqc * 512, (sqc + 1) * 512)
                ot_sb = opool.tile([128, 2, 512], F16, name="ot", tag="ot")
                for hp in range(2):  # head pair = fc chunk
                    o_ps = [
                        ps_o.tile([D + 1, 512], F32, name="oacc", tag="oacc")
                        for _ in range(2)
                    ]
                    p_tiles = {}
                    for skc in range(SKN + 1):
                        if skc < SKN:
                            sk = slice(skc * 128, (skc + 1) * 128)
                            s_ps = ps_s.tile([128, 1024], F32, name="sacc",
                                             tag="sacc")
                            # two heads row-packed into the 128-deep array
                            for h2 in range(2):
                                rows = slice(h2 * 64, (h2 + 1) * 64)
                                nc.tensor.matmul(
                                    out=s_ps[:, h2 * 512:(h2 + 1) * 512],
                                    lhsT=kt_sb[rows, hp, sk],
                                    rhs=qt_sb[rows, hp, sq],
                                    start=True,
                                    stop=True,
                                )
                            p_sb = ppool.tile([128, 1024], F16, name="p",
                                              tag="p")
                            nc.scalar.activation(
                                out=p_sb,
                                in_=s_ps[:, :],
                                func=mybir.ActivationFunctionType.Exp,
                                scale=SCALE,
                            )
                            p_tiles[skc] = p_sb
                        if skc >= 1:
                            pk = skc - 1
                            p_sb = p_tiles.pop(pk)
                            for h2 in range(2):
                                nc.tensor.matmul(
                                    out=o_ps[h2],
                                    lhsT=vp_sb[:, pk, hp * 2 + h2, :],
                                    rhs=p_sb[:, h2 * 512:(h2 + 1) * 512],
                                    start=(pk == 0),
                                    stop=(pk == SKN - 1),
                                )
                        if pending and skc >= 1 and (skc + hp) % 2 == 1:
                            pending.pop(0)()
                    # normalize: rows 0..63 = O^T, row 64 = sum(exp).
                    # Copy out of PSUM first (early bank release), then
                    # broadcast the denominator row and divide on DVE.
                    last_pair = (sqc == SQN - 1 and hp == 1)
                    for h2 in range(2):
                        o_sb = small.tile([D + 1, 512], F32, name="osb",
                                          tag="osb")
                        if last_pair and h2 == 1:
                            nc.scalar.copy(out=o_sb, in_=o_ps[h2])
                        else:
                            nc.vector.tensor_copy(out=o_sb, in_=o_ps[h2])
                        den0 = small.tile([1, 512], F32, name="den0",
                                          tag="den0")
                        # move the denominator row to partition 0 (DMA can
                        # cross partitions; DVE ops must stay aligned)
                        nc.sync.dma_start(out=den0, in_=o_sb[D:D + 1, :])
                        rec = small.tile([1, 512], F32, name="rec",
                                         tag="rec")
                        nc.vector.reciprocal_approx_fast(out=rec, in_=den0)
                        rec_b = small.tile([D, 512], F32, name="recb",
                                           tag="recb")
                        nc.gpsimd.partition_broadcast(rec_b, rec)
                        if h2 == 0:
                            nc.vector.tensor_mul(
                                out=ot_sb[0:D, hp, :],
                                in0=o_sb[0:D, :],
                                in1=rec_b,
                            )
                        else:
                            tmp = small.tile([D, 512], F16, name="otmp",
                                             tag="otmp")
                            nc.vector.tensor_mul(
                                out=tmp, in0=o_sb[0:D, :], in1=rec_b
                            )
                            # cross-partition move (DVE lanes can't shift)
                            nc.sync.dma_start(out=ot_sb[D:128, hp, :], in_=tmp)

                pending = [step for oc in range(8)
                           for step in yproj_steps(
                               ot_sb, sq, oc, tail=(sqc == SQN - 1))]
            for step in pending:
                step()


def build_bass():
    nc = bacc.Bacc("TRN2", target_bir_lowering=False, debug=False,
                   enable_asserts=False)
    qT = nc.dram_tensor("qT", [C, SEQ], F16, kind="ExternalInput").ap()
    kT = nc.dram_tensor("kT", [C, SEQ], F16, kind="ExternalInput").ap()
    vT = nc.dram_tensor("vT", [C, SEQ], F16, kind="ExternalInput").ap()
    wq = nc.dram_tensor("wq", [C, FH], F16, kind="ExternalInput").ap()
    wk = nc.dram_tensor("wk", [C, FH], F16, kind="ExternalInput").ap()
    wv = nc.dram_tensor("wv", [C, FH], F16, kind="ExternalInput").ap()
    wo = nc.dram_tensor("wo", [FH, C], F16, kind="ExternalInput").ap()
    yT = nc.dram_tensor("yT", [C, SEQ], F32, kind="ExternalOutput").ap()
    with tile.TileContext(nc) as tc:
        build_kernel(tc, qT, kT, vT, wq, wk, wv, wo, yT)
    nc.compile()
    return nc


_NC = None


def _get_nc():
    global _NC
    if _NC is None:
        _NC = build_bass()
    return _NC


def make_in_maps(q, k, v, Wq, Wk, Wv, Wo):
    f16 = np.float16
    in_maps = []
    for c in range(N_CORES):
        b, g = divmod(c, 4)
        fs = slice(g * FH, (g + 1) * FH)
        in_maps.append(dict(
            qT=np.ascontiguousarray(q[b].T).astype(f16),
            kT=np.ascontiguousarray(k[b].T).astype(f16),
            vT=np.ascontiguousarray(v[b].T).astype(f16),
            wq=np.ascontiguousarray(Wq[:, fs]).astype(f16),
            wk=np.ascontiguousarray(Wk[:, fs]).astype(f16),
            wv=np.ascontiguousarray(Wv[:, fs]).astype(f16),
            wo=np.ascontiguousarray(Wo[fs, :]).astype(f16),
        ))
    return in_maps


def kernel(q, k, v, Wq, Wk, Wv, Wo, bo):
    global LAST_RESULTS
    q = np.asarray(q, dtype=np.float32)
    k = np.asarray(k, dtype=np.float32)
    v = np.asarray(v, dtype=np.float32)
    Wq = np.asarray(Wq, dtype=np.float32)
    Wk = np.asarray(Wk, dtype=np.float32)
    Wv = np.asarray(Wv, dtype=np.float32)
    Wo = np.asarray(Wo, dtype=np.float32)
    bo = np.asarray(bo, dtype=np.float32)

    nc = _get_nc()
    in_maps = make_in_maps(q, k, v, Wq, Wk, Wv, Wo)
    res = run_bass_kernel_spmd(
        nc, in_maps, list(range(N_CORES)),
        trace=bool(os.environ.get("KERNEL_TRACE")),
    )
    LAST_RESULTS = res

    out = np.zeros((B, SEQ, C), dtype=np.float32)
    for c in range(N_CORES):
        out[c // 4] += res.results[c]["yT"].T
    out += bo
    return out.astype(np.float32)
